# revision 2
# baseline (speedup 1.0000x reference)
"""Trainium2 Bass kernel for a BinaryNet conv block (v2).

Pipeline (per core, data-parallel over batch):
  sign(x) -> conv3x3(sign(w1)) -> BN1 -> sign -> conv3x3(sign(w2))
          -> maxpool2x2 -> BN2

v2 changes vs baseline:
  - Input transposes run as regular fp8 DoubleRow matmuls (data stationary,
    block-diagonal identity moving): 0.5 cyc/col instead of 1.0.
  - Output stays channel-major on device; host does the final transpose.
    (kills all output PE transposes + the gather copies)
  - Conv stretches are <=8 rows (<=464 cols, one matmul per tap, single
    PSUM bank, no 512-col splits).
  - y is bf16 (halves store bytes); host upcasts.
  - sign(x) is split across DVE (chunk0, +-0.5) and ACT (chunk1, +-1 via
    Sign activation); w1 chunk0 is pre-scaled x2 so conv1 PSUM = h1 exactly.
  - border memsets on GPSIMD; pool max1 on DVE directly from PSUM.
  - img0's x load is split into pieces so conv1 can start ~6us in.
"""

import os
import numpy as np

os.environ.setdefault("MYCRO_LOCAL_CACHE", "1")

N_CORES = 8
C = 256
NCHUNK = 2
KP = 128

# packed consts layout (bytes per partition); cb1 = [0, CB1), cb2 = rest
W1_OFF = 0
NT1_OFF = 4608  # f32 [2]
S2_OFF = 4616
B2_OFF = 4624
CB1 = 4632
W2_OFF = 4632
CONST_B = 9240

# img0 x-load pieces, in blocks (2 rows each); must align to PGROUP
X0_PIECES = (7, 7, 7, 7)
USE_GPSIMD_CB = False  # consts via SWDGE (Pool-engine) DMA, off the HWDGE path
SPLIT_Y3 = True  # ship img3 output in two pieces to shorten the tail
WARMUP_MM = 45  # dummy PE matmuls to climb the p-state ramp during startup
PGROUP = 7  # sign-group size in blocks

# conv row-stretches (r0, rg): rg*Wp <= 512 psum bank; even rg for pooling
STRETCHES = [(0, 8), (8, 4), (12, 8), (20, 8), (28, 8), (36, 4), (40, 8), (48, 8)]


def build_program(B, H, W):
    """Build the per-core Bass program. B images of HxWxC per core."""
    import concourse.bass as bass
    import concourse.bacc as bacc
    import concourse.tile as tile
    from concourse import mybir, masks

    F32 = mybir.dt.float32
    FP16 = mybir.dt.float16
    BF16 = mybir.dt.bfloat16
    FP8 = mybir.dt.float8e4
    U8 = mybir.dt.uint8
    DR = mybir.MatmulPerfMode.DoubleRow
    Alu = mybir.AluOpType
    Act = mybir.ActivationFunctionType

    Hp, Wp = H + 2, W + 2
    DOFF = 32  # left zero pad inside each channel-chunk row buffer
    S_chunk = ((Hp * Wp + DOFF + 32 + 15) // 16) * 16
    RB = 2 * W  # transpose block = 2 image rows
    NB = H // 2  # blocks per image
    PO = (H // 2) * (W // 2)
    WH = W // 2
    HH = H // 2
    TPB = 4  # transpose-psum blocks per tile (512B slots -> bank aligned)

    assert sum(rg for _, rg in STRETCHES) == H
    assert all(rg * Wp <= 512 for _, rg in STRETCHES)
    assert all(rg % 2 == 0 for _, rg in STRETCHES)

    nc = bacc.Bacc("TRN2", target_bir_lowering=False, debug=False)

    x_h = nc.dram_tensor("x", [B, H * W, C], F32, kind="ExternalInput")
    cb_h = nc.dram_tensor("cb", [KP, CONST_B], U8, kind="ExternalInput")
    y_h = nc.dram_tensor("y", [B, NCHUNK, KP, PO], BF16, kind="ExternalOutput")

    def dram_ap(handle, offset, dims):
        return bass.AP(
            tensor=handle.ap().tensor, offset=offset, ap=[list(d) for d in dims]
        )

    with tile.TileContext(nc) as tc:
        from contextlib import ExitStack

        with ExitStack() as ctx:
            consts = ctx.enter_context(tc.tile_pool(name="consts", bufs=1))
            xnat_p = ctx.enter_context(tc.tile_pool(name="xnat", bufs=1))
            xsg_p = ctx.enter_context(tc.tile_pool(name="xsg", bufs=2))
            xsT_p = ctx.enter_context(tc.tile_pool(name="xsT", bufs=2))
            hsT_p = ctx.enter_context(tc.tile_pool(name="hsT", bufs=2))
            prq_p = ctx.enter_context(tc.tile_pool(name="prq", bufs=2))
            pv_p = ctx.enter_context(tc.tile_pool(name="pvp", bufs=2))
            onat_p = ctx.enter_context(tc.tile_pool(name="onat", bufs=1))
            convp = ctx.enter_context(tc.tile_pool(name="convp", bufs=4, space="PSUM"))
            tp_p = ctx.enter_context(tc.tile_pool(name="tpp", bufs=2, space="PSUM"))

            # --- identities for the DR-matmul transposes, built on GPSIMD
            ident = consts.tile([KP, NCHUNK, 2 * RB], FP8)
            nc.gpsimd.memset(ident, 0.0)
            for r in range(NCHUNK):
                masks.make_identity(
                    nc, ident[:RB, r, RB * r : RB * (r + 1)], nomemset=True
                )

            # --- packed constants: one DMA; bitcast views
            cb = consts.tile([KP, CONST_B], U8)

            def load_consts1():
                nc.sync.dma_start(
                    out=cb[:, 0:CB1],
                    in_=dram_ap(cb_h, 0, [[CONST_B, KP], [1, CB1]]),
                )

            def load_consts2():
                nc.sync.dma_start(
                    out=cb[:, CB1:CONST_B],
                    in_=dram_ap(cb_h, CB1, [[CONST_B, KP], [1, CONST_B - CB1]]),
                )

            w1sb = cb[:, W1_OFF : W1_OFF + 4608].bitcast(FP8).rearrange(
                "p (t j k m) -> p t j k m", t=9, j=NCHUNK, k=2
            )
            w2sb = cb[:, W2_OFF : W2_OFF + 4608].bitcast(FP8).rearrange(
                "p (t j k m) -> p t j k m", t=9, j=NCHUNK, k=2
            )
            nt1sb = cb[:, NT1_OFF : NT1_OFF + 8].bitcast(F32)
            s2sb = cb[:, S2_OFF : S2_OFF + 8].bitcast(F32)
            b2sb = cb[:, B2_OFF : B2_OFF + 8].bitcast(F32)

            def border_memsets(buf):
                nc.gpsimd.memset(buf[:, :, 0 : DOFF + Wp], 0.0)
                nc.gpsimd.memset(buf[:, :, DOFF + (H + 1) * Wp : S_chunk], 0.0)
                rows = buf[:, :, DOFF + Wp : DOFF + (H + 1) * Wp].rearrange(
                    "p j (r w) -> p j r w", w=Wp
                )
                nc.gpsimd.memset(rows[:, :, :, 0 :: (W + 1)], 0.0)

            # ------------------------------------------------------------------
            # input prep: sign -> DR-matmul transpose -> scatter into padded
            # channel-major fp8 layout
            # ------------------------------------------------------------------
            xsT_tiles = {}

            def prep_group(img, g, b0g, xn_view, nblk, xsT):
                """sign+transpose+scatter for blocks [b0g, b0g+nblk)."""
                xg = xsg_p.tile([RB, PGROUP, C], FP8, tag="xg", name=f"xg{img}{g}")
                # chunk0 on DVE (+-0.5), chunk1 on ACT (+-1; w1 compensates)
                nc.vector.tensor_scalar(
                    xg[:, :nblk, 0:KP], xn_view[:, :, 0:KP], 0.0, 0.5,
                    Alu.is_ge, Alu.subtract,
                )
                nc.scalar.activation(
                    xg[:, :nblk, KP:C], xn_view[:, :, KP:C], Act.Sign,
                    bias=0.0, scale=1.0,
                )
                for t0 in range(0, nblk, TPB):
                    tn = min(TPB, nblk - t0)
                    tp = tp_p.tile(
                        [KP, TPB, 256], F32, tag="tp", name=f"tp{img}{g}{t0}"
                    )
                    for bi in range(tn):
                        lhsT = xg[:, t0 + bi, :].rearrange("p (k m) -> p k m", k=2)
                        nc.tensor.matmul(
                            tp[:, bi, 0 : 2 * RB],
                            lhsT,
                            ident[:RB],
                            start=True,
                            stop=True,
                            perf_mode=DR,
                        )
                    # scatter: tp[p, bi, RB*j + 56*r + c] -> xsT rows
                    b0 = b0g + t0
                    for j in range(NCHUNK):
                        src = tp[:, 0:tn, RB * j : RB * j + RB].rearrange(
                            "p b (r w) -> p b r w", w=W
                        )
                        a0 = DOFF + (1 + 2 * b0) * Wp
                        dst = xsT[:, j, a0 : a0 + 2 * tn * Wp].rearrange(
                            "p (b r w) -> p b r w", r=2, w=Wp
                        )[:, :, :, 1 : 1 + W]
                        if j == 0:
                            nc.scalar.copy(dst, src)
                        else:
                            nc.vector.tensor_copy(dst, src)

            def prep_input(img, pieces):
                """pieces: list of (xn_view_fn, nblk) DMA'd natural tiles."""
                xsT = xsT_p.tile(
                    [KP, NCHUNK, S_chunk], FP8, tag="xsT", name=f"xsT{img}"
                )
                border_memsets(xsT)
                g = 0
                for xn, nblk_piece in pieces:
                    off = 0
                    while off < nblk_piece:
                        nblk = min(PGROUP, nblk_piece - off)
                        prep_group(img, g, xn[:, off : off + nblk, :], nblk, xsT)
                        off += nblk
                        g += 1
                xsT_tiles[img] = xsT

            # ------------------------------------------------------------------
            # conv as 9 shifted DR matmuls per (stretch, chunk)
            # ------------------------------------------------------------------
            def conv_stretch(inbuf, wsb, si, psum_cb, cv):
                # per-row matmuls (56 wide) skip the 2 pad columns per row;
                # row 0 of tap 0 carries start (marks the whole psum zero
                # region), the last row of tap 8 carries stop
                r0, rg = STRETCHES[si]
                cs = (1 + r0) * Wp
                for j in range(NCHUNK):
                    ps = convp.tile([KP, 512], F32, tag="cv", name=f"{cv}{si}{j}")
                    for t in range(9):
                        dy, dx = t // 3, t % 3
                        off = (dy - 1) * Wp + (dx - 1)
                        a = DOFF + cs + off + 1
                        for r in range(rg):
                            nc.tensor.matmul(
                                ps[:, r * Wp + 1 : r * Wp + 1 + W],
                                wsb[:, t, j],
                                inbuf[:, :, a + r * Wp : a + r * Wp + W],
                                start=(t == 0 and r == 0),
                                stop=(t == 8 and r == rg - 1),
                                perf_mode=DR,
                            )
                    psum_cb(si, j, ps, r0, rg)

            onat_box = [None]

            def make_prep(img, pieces):
                """Allocate xsT(img) + memsets; return per-group emit closures."""
                xsT = xsT_p.tile(
                    [KP, NCHUNK, S_chunk], FP8, tag="xsT", name=f"xsT{img}"
                )
                border_memsets(xsT)
                closures = []
                g = 0
                b0g = 0
                for xn, nblk_piece in pieces:
                    off = 0
                    while off < nblk_piece:
                        nblk = min(PGROUP, nblk_piece - off)

                        def mk(g=g, b0g=b0g, xn=xn, off=off, nblk=nblk):
                            return lambda: prep_group(
                                img, g, b0g, xn[:, off : off + nblk, :], nblk, xsT
                            )

                        closures.append(mk())
                        off += nblk
                        b0g += nblk
                        g += 1
                xsT_tiles[img] = xsT
                return closures

            def conv1_emitters(img):
                xsT = xsT_tiles.pop(img)
                hsT = hsT_p.tile(
                    [KP, NCHUNK, S_chunk], FP8, tag="hsT", name=f"hsT{img}"
                )
                border_memsets(hsT)

                def bnsign(si, j, ps, r0, rg):
                    a0 = DOFF + (1 + r0) * Wp
                    dstv = hsT[:, j, a0 : a0 + rg * Wp].rearrange(
                        "p (r w) -> p r w", w=Wp
                    )[:, :, 1 : 1 + W]
                    srcv = ps[:, 0 : rg * Wp].rearrange("p (r w) -> p r w", w=Wp)[
                        :, :, 1 : 1 + W
                    ]
                    nc.scalar.activation(
                        dstv, srcv, Act.Sign, bias=nt1sb[:, j : j + 1], scale=1.0
                    )

                def emit(si):
                    conv_stretch(xsT, w1sb, si, bnsign, f"c1i{img}s")

                return hsT, emit

            def conv1_image(img, own_groups=None):
                hsT, emit = conv1_emitters(img)
                for si in range(len(STRETCHES)):
                    if own_groups and si in own_groups:
                        own_groups[si]()
                    emit(si)
                return hsT

            def conv2_emitters(img, hsT):
                if img == 0:
                    onat_box[0] = onat_p.tile(
                        [KP, 3, NCHUNK, PO], BF16, tag="on012", name="on012"
                    )
                if img == 3:
                    onat_box[0] = onat_p.tile(
                        [KP, 1, NCHUNK, PO], BF16, tag="on3", name="on3"
                    )
                onat = onat_box[0]
                slot = img if img < 3 else 0

                def pool(si, j, ps, r0, rg):
                    q0, q = r0 // 2, rg // 2
                    # full 2x2 maxpool in one DVE reduce (one PSUM operand),
                    # fp16 holds conv2 integers exactly
                    win = ps[:, 0 : rg * Wp].rearrange(
                        "p (q r w) -> p q r w", r=2, w=Wp
                    )[:, :, :, 1 : 1 + W]
                    win = win.rearrange("p q r (u c) -> p q u r c", c=2)
                    pv = pv_p.tile(
                        [KP, 4, WH], FP16, tag="pv", bufs=2, name=f"pv{img}{si}{j}"
                    )
                    nc.vector.tensor_reduce(
                        pv[:, 0:q, :], win, mybir.AxisListType.XY, Alu.max
                    )
                    pvv = pv[:, 0:q, :].rearrange("p q w -> p (q w)")
                    nc.vector.tensor_scalar(
                        onat[:, slot, j, q0 * WH : (q0 + q) * WH],
                        pvv,
                        s2sb[:, j : j + 1],
                        b2sb[:, j : j + 1],
                        Alu.mult,
                        Alu.add,
                    )

                def emit(si):
                    conv_stretch(hsT, w2sb, si, pool, f"c2i{img}s")
                    if img == 3 and SPLIT_Y3 and si == len(STRETCHES) - 3:
                        r0l, rgl = STRETCHES[-2]
                        po_cut = (r0l // 2) * WH
                        dst = dram_ap(
                            y_h,
                            3 * NCHUNK * KP * PO,
                            [[PO, KP], [KP * PO, NCHUNK], [1, po_cut]],
                        )
                        nc.sync.dma_start(out=dst, in_=onat[:, 0, :, 0:po_cut])

                def finish():
                    if img == 2:
                        dst = dram_ap(
                            y_h,
                            0,
                            [
                                [PO, KP],
                                [NCHUNK * KP * PO, 3],
                                [KP * PO, NCHUNK],
                                [1, PO],
                            ],
                        )
                        nc.sync.dma_start(out=dst, in_=onat[:, :, :, :])
                    if img == 3:
                        po_cut = (
                            (STRETCHES[-2][0] // 2) * WH if SPLIT_Y3 else 0
                        )
                        dst = dram_ap(
                            y_h,
                            3 * NCHUNK * KP * PO + po_cut,
                            [[PO, KP], [KP * PO, NCHUNK], [1, PO - po_cut]],
                        )
                        nc.sync.dma_start(out=dst, in_=onat[:, 0, :, po_cut:PO])

                return emit, finish

            def conv2_image(img, hsT, next_groups):
                emit, finish = conv2_emitters(img, hsT)
                for si in range(len(STRETCHES)):
                    if si in next_groups:
                        next_groups[si]()
                    emit(si)
                for k in sorted(next_groups):
                    if k >= len(STRETCHES):
                        next_groups[k]()
                finish()

            # ------------------------------------------------------------------
            # emission: all DMAs up front; img0's prep interleaved into its own
            # conv1; prep(i+1) interleaved into conv2(i)
            # ------------------------------------------------------------------
            piece_tiles = []
            b0 = 0
            for pi, nblk in enumerate(X0_PIECES):
                t = xnat_p.tile(
                    [RB, nblk, C], F32, tag=f"xn0{pi}", name=f"xn0{pi}"
                )
                nc.sync.dma_start(
                    out=t,
                    in_=dram_ap(
                        x_h, b0 * RB * C, [[C, RB], [RB * C, nblk], [1, C]]
                    ),
                )
                piece_tiles.append((t, nblk))
                b0 += nblk
                if pi == 0:
                    load_consts1()
            load_consts2()
            xn1 = xnat_p.tile([RB, NB, C], F32, tag="xn1", name="xn1")
            nc.sync.dma_start(
                out=xn1,
                in_=dram_ap(x_h, H * W * C, [[C, RB], [RB * C, NB], [1, C]]),
            )
            xn23 = xnat_p.tile([RB, 2 * NB, C], F32, tag="xn23", name="xn23")
            nc.sync.dma_start(
                out=xn23,
                in_=dram_ap(
                    x_h, 2 * H * W * C, [[C, RB], [RB * C, 2 * NB], [1, C]]
                ),
            )
            xn_of = {1: xn1, 2: xn23[:, :NB, :], 3: xn23[:, NB:, :]}

            # conv1 stretch si needs sign-groups: s0,s1: g0; s2: g1; s3..s5: g2
            # (s3 reads row 28); s6,s7: g3
            # warm the PE p-state ramp with dummy transposes while the
            # first x piece + sign are still in flight
            warm = tp_p.tile([KP, TPB, 256], F32, tag="tp", name="warm")
            for wi in range(WARMUP_MM):
                nc.tensor.matmul(
                    warm[:, wi % TPB, 0 : 2 * RB],
                    ident[:RB, :, 0:KP],
                    ident[:RB],
                    start=True,
                    stop=True,
                    perf_mode=DR,
                )

            # img0: interleave conv1/conv2 stretches (conv2 lags conv1 by
            # one stretch) so conv2 work fills the x-piece feed stalls
            prep0 = make_prep(0, piece_tiles)
            own0 = {0: prep0[0], 1: prep0[1], 3: prep0[2], 5: prep0[3]}
            hsT, c1emit = conv1_emitters(0)
            c2emit, c2fin = conv2_emitters(0, hsT)
            prep1 = make_prep(1, [(xn_of[1], NB)])
            ng1 = {4: prep1[0], 5: prep1[1], 6: prep1[2], 7: prep1[3]}
            NS = len(STRETCHES)
            for si in range(NS):
                if si in own0:
                    own0[si]()
                c1emit(si)
                if si >= 1:
                    if (si - 1) in ng1:
                        ng1[si - 1]()
                    c2emit(si - 1)
            if (NS - 1) in ng1:
                ng1[NS - 1]()
            c2emit(NS - 1)
            c2fin()
            hsT = conv1_image(1)
            for img in range(1, B):
                if img + 1 < B:
                    prepn = make_prep(img + 1, [(xn_of[img + 1], NB)])
                    ng = {2: prepn[0], 3: prepn[1], 5: prepn[2], 7: prepn[3]}
                else:
                    ng = {}
                conv2_image(img, hsT, ng)
                if img + 1 < B:
                    hsT = conv1_image(img + 1)

    nc.compile()
    return nc


# ---------------------------------------------------------------------------
# host-side constant prep
# ---------------------------------------------------------------------------


def _prep_consts(w1, beta1, mean1, var1, w2, beta2, mean2, var2):
    import jax
    import jax.numpy as jnp
    from jax import lax
    from concourse import mybir

    fp8np = mybir.dt.np(mybir.dt.float8e4)

    def prep_w(w, k0_scale, both=False):
        ws = np.where(np.asarray(w) >= 0, np.float32(1.0), np.float32(-1.0))
        # [3,3,ci,co] -> [p, tap, j, ktile, m]; ci = ktile*128+p, co = j*128+m
        wr = ws.reshape(9, 2, KP, NCHUNK, KP).transpose(2, 0, 3, 1, 4).copy()
        if both:
            wr *= k0_scale
        else:
            wr[:, :, :, 0, :] *= k0_scale  # compensate x-chunk0 magnitude
        return np.ascontiguousarray(wr).astype(fp8np)

    # conv1: x = +-0.5 everywhere -> scale all conv1 weights x2
    w1p = prep_w(w1, 2.0)
    # conv2: both chunks +-1
    w2p = prep_w(w2, 1.0)

    cpu = jax.devices("cpu")[0]
    MAXH = 9 * C
    with jax.default_device(cpu):
        hs = jnp.arange(-MAXH, MAXH + 1, dtype=jnp.float32)
        bn1 = (hs[:, None] - jnp.asarray(mean1)[None, :]) * lax.rsqrt(
            jnp.asarray(var1) + 1e-3
        )[None, :] + jnp.asarray(beta1)[None, :]
        nonneg = np.asarray(bn1 >= 0)
        r2 = np.asarray(lax.rsqrt(jnp.asarray(var2) + 1e-3))

    assert (np.diff(nonneg.astype(np.int8), axis=0) >= 0).all(), "bn1 not monotone"
    kc = np.where(nonneg.any(0), nonneg.argmax(0), 2 * MAXH + 1) - MAXH
    # device psum holds h1 exactly: sign flips at kc - 0.5
    nt1 = (-(kc.astype(np.float64) - 0.5)).astype(np.float32)

    s2 = r2.astype(np.float32)
    b2 = (
        np.asarray(beta2, np.float64)
        - np.asarray(mean2, np.float64) * s2.astype(np.float64)
    ).astype(np.float32)

    def to_pj(a):  # [256] -> [128, 2] with c = j*128+p
        return np.ascontiguousarray(a.reshape(NCHUNK, KP).T).astype(np.float32)

    cbuf = np.zeros((KP, CONST_B), dtype=np.uint8)

    def put(off, arr):
        by = np.ascontiguousarray(arr).reshape(KP, -1).view(np.uint8)
        cbuf[:, off : off + by.shape[1]] = by

    put(W1_OFF, w1p)
    put(W2_OFF, w2p)
    put(NT1_OFF, to_pj(nt1))
    put(S2_OFF, to_pj(s2))
    put(B2_OFF, to_pj(b2))
    return {"cb": cbuf}


# ---------------------------------------------------------------------------
# entry point
# ---------------------------------------------------------------------------

_cached = {}


def _get_program(Bc, H, W):
    key = (Bc, H, W)
    if key not in _cached:
        _cached[key] = build_program(Bc, H, W)
    return _cached[key]


def _unshard(res_list, Bt, H, W):
    # y per core: [Bc, 2, 128, PO] bf16 -> [Bt, H/2, W/2, C] f32
    y = np.concatenate(
        [np.asarray(r["y"]).astype(np.float32) for r in res_list], axis=0
    )
    y = y.transpose(0, 3, 1, 2).reshape(Bt, (H // 2) * (W // 2), C)
    return y.reshape(Bt, H // 2, W // 2, C)


def _run(inputs, trace=False):
    from concourse import bass_utils

    x = np.asarray(inputs["x"], dtype=np.float32)
    Bt, H, W, _ = x.shape  # 32, 56, 56, 256
    Bc = Bt // N_CORES

    consts = _prep_consts(
        inputs["w1"], inputs["beta1"], inputs["mean1"], inputs["var1"],
        inputs["w2"], inputs["beta2"], inputs["mean2"], inputs["var2"],
    )

    nc = _get_program(Bc, H, W)

    in_maps = []
    for c in range(N_CORES):
        m = dict(consts)
        m["x"] = np.ascontiguousarray(x[c * Bc : (c + 1) * Bc].reshape(Bc, H * W, C))
        in_maps.append(m)

    res = bass_utils.run_bass_kernel_spmd(
        nc, in_maps, core_ids=list(range(N_CORES)), trace=trace
    )
    y = _unshard(res.results, Bt, H, W)
    return y, res


def kernel(**inputs):
    y, _ = _run(inputs, trace=False)
    return y


# revision 3
# speedup vs baseline: 1.0338x; 1.0338x over previous
"""Trainium2 Bass kernel for a BinaryNet conv block.

Pipeline (per core, data-parallel over batch across 8 cores):
  sign(x) -> conv3x3(sign(w1)) -> BN1 -> sign -> conv3x3(sign(w2))
          -> maxpool2x2 -> BN2

Design notes (see TimelineSim cost model for the numbers):
  - Convs run as 9-tap shifted matmuls in fp8 DoubleRow (K=256/pass,
    0.5 cyc per output column). Matmuls are emitted per image row (56
    cols) so the 2 pad columns per 58-wide padded row are never
    computed; row 0 of tap 0 carries `start`, last row of tap 8 `stop`.
  - Activations are +-0.5 (chunk0, DVE tensor_scalar) / +-1 (chunk1, ACT
    Sign); w1's chunk-0 rows are pre-scaled x2 so conv1 PSUM = h1
    exactly, and BN1+sign folds into one ACT Sign against an integer
    threshold (bit-exact vs the fp32 reference).
  - Input transposes into the channel-major layout run as regular fp8
    DoubleRow matmuls: the signed 2-row block is the *stationary*
    operand and a block-diagonal [112, 2, 224] identity streams, giving
    0.5 cyc/col (half the cost of transpose-mode matmuls).
  - conv2 output is 2x2-maxpooled by a single DVE tensor_reduce
    (axis=XY) per stretch straight out of PSUM (engines may read only
    one PSUM operand per instruction), then BN2 via tensor_scalar into
    a bf16 channel-major output tile; the host does the final
    [b, c, s] -> [b, s, c] transpose and f32 upcast (bf16 rounding is
    ~3e-3 relative, well inside the 2e-2 gate).
  - Conv stretches are <=8 rows so each (stretch, chunk) PSUM group
    fits one 2KB bank; PSUM: 4 conv bufs + 2 transpose bufs = 8 banks.
  - Schedule: the Tile list-scheduler follows emission order among
    ready ops, so emission is the schedule: img0's x load is split in
    four pieces with sign/transpose/scatter groups interleaved into
    conv1 stretches, conv2(img0) lags conv1 by two stretches to fill
    the DMA feed stalls, and prep(img i+1) is interleaved into
    conv2(img i). Consts load in two pieces (w1 early, w2 late); the
    img3 output ships in three pieces so the tail only waits on the
    last 224 columns.
"""

import os
import numpy as np

os.environ.setdefault("MYCRO_LOCAL_CACHE", "1")

N_CORES = 8
C = 256
NCHUNK = 2
KP = 128

# packed consts layout (bytes per partition); cb1 = [0, CB1), cb2 = rest
W1_OFF = 0
NT1_OFF = 4608  # f32 [2]
S2_OFF = 4616
B2_OFF = 4624
CB1 = 4632
W2_OFF = 4632
CONST_B = 9240

# img0 x-load pieces, in blocks (2 rows each); must align to PGROUP
X0_PIECES = (7, 7, 7, 7)
USE_GPSIMD_CB = False  # consts via SWDGE (Pool-engine) DMA, off the HWDGE path
SPLIT_Y3 = True  # ship img3 output in two pieces to shorten the tail
WARMUP_MM = 45  # dummy PE matmuls to climb the p-state ramp during startup
PGROUP = 7  # sign-group size in blocks

# conv row-stretches (r0, rg): rg*Wp <= 512 psum bank; even rg for pooling
STRETCHES = [(0, 8), (8, 4), (12, 8), (20, 8), (28, 8), (36, 4), (40, 8), (48, 8)]


def build_program(B, H, W):
    """Build the per-core Bass program. B images of HxWxC per core."""
    import concourse.bass as bass
    import concourse.bacc as bacc
    import concourse.tile as tile
    from concourse import mybir, masks

    F32 = mybir.dt.float32
    FP16 = mybir.dt.float16
    BF16 = mybir.dt.bfloat16
    FP8 = mybir.dt.float8e4
    U8 = mybir.dt.uint8
    DR = mybir.MatmulPerfMode.DoubleRow
    Alu = mybir.AluOpType
    Act = mybir.ActivationFunctionType

    Hp, Wp = H + 2, W + 2
    DOFF = 32  # left zero pad inside each channel-chunk row buffer
    S_chunk = ((Hp * Wp + DOFF + 32 + 15) // 16) * 16
    RB = 2 * W  # transpose block = 2 image rows
    NB = H // 2  # blocks per image
    PO = (H // 2) * (W // 2)
    WH = W // 2
    HH = H // 2
    TPB = 4  # transpose-psum blocks per tile (512B slots -> bank aligned)

    assert sum(rg for _, rg in STRETCHES) == H
    assert all(rg * Wp <= 512 for _, rg in STRETCHES)
    assert all(rg % 2 == 0 for _, rg in STRETCHES)

    nc = bacc.Bacc("TRN2", target_bir_lowering=False, debug=False)

    x_h = nc.dram_tensor("x", [B, H * W, C], F32, kind="ExternalInput")
    cb_h = nc.dram_tensor("cb", [KP, CONST_B], U8, kind="ExternalInput")
    y_h = nc.dram_tensor("y", [B, NCHUNK, KP, PO], BF16, kind="ExternalOutput")

    def dram_ap(handle, offset, dims):
        return bass.AP(
            tensor=handle.ap().tensor, offset=offset, ap=[list(d) for d in dims]
        )

    with tile.TileContext(nc) as tc:
        from contextlib import ExitStack

        with ExitStack() as ctx:
            consts = ctx.enter_context(tc.tile_pool(name="consts", bufs=1))
            xnat_p = ctx.enter_context(tc.tile_pool(name="xnat", bufs=1))
            xsg_p = ctx.enter_context(tc.tile_pool(name="xsg", bufs=3))
            xsT_p = ctx.enter_context(tc.tile_pool(name="xsT", bufs=2))
            hsT_p = ctx.enter_context(tc.tile_pool(name="hsT", bufs=3))
            pv_p = ctx.enter_context(tc.tile_pool(name="pvp", bufs=2))
            onat_p = ctx.enter_context(tc.tile_pool(name="onat", bufs=1))
            convp = ctx.enter_context(tc.tile_pool(name="convp", bufs=4, space="PSUM"))
            tp_p = ctx.enter_context(tc.tile_pool(name="tpp", bufs=2, space="PSUM"))

            # --- identities for the DR-matmul transposes, built on GPSIMD
            ident = consts.tile([KP, NCHUNK, 2 * RB], FP8)
            nc.gpsimd.memset(ident, 0.0)
            for r in range(NCHUNK):
                masks.make_identity(
                    nc, ident[:RB, r, RB * r : RB * (r + 1)], nomemset=True
                )

            # --- packed constants: one DMA; bitcast views
            cb = consts.tile([KP, CONST_B], U8)

            def load_consts1():
                nc.sync.dma_start(
                    out=cb[:, 0:CB1],
                    in_=dram_ap(cb_h, 0, [[CONST_B, KP], [1, CB1]]),
                )

            def load_consts2():
                nc.sync.dma_start(
                    out=cb[:, CB1:CONST_B],
                    in_=dram_ap(cb_h, CB1, [[CONST_B, KP], [1, CONST_B - CB1]]),
                )

            w1sb = cb[:, W1_OFF : W1_OFF + 4608].bitcast(FP8).rearrange(
                "p (t j k m) -> p t j k m", t=9, j=NCHUNK, k=2
            )
            w2sb = cb[:, W2_OFF : W2_OFF + 4608].bitcast(FP8).rearrange(
                "p (t j k m) -> p t j k m", t=9, j=NCHUNK, k=2
            )
            nt1sb = cb[:, NT1_OFF : NT1_OFF + 8].bitcast(F32)
            s2sb = cb[:, S2_OFF : S2_OFF + 8].bitcast(F32)
            b2sb = cb[:, B2_OFF : B2_OFF + 8].bitcast(F32)

            def border_memsets(buf):
                nc.gpsimd.memset(buf[:, :, 0 : DOFF + Wp], 0.0)
                nc.gpsimd.memset(buf[:, :, DOFF + (H + 1) * Wp : S_chunk], 0.0)
                rows = buf[:, :, DOFF + Wp : DOFF + (H + 1) * Wp].rearrange(
                    "p j (r w) -> p j r w", w=Wp
                )
                nc.gpsimd.memset(rows[:, :, :, 0 :: (W + 1)], 0.0)

            # ------------------------------------------------------------------
            # input prep: sign -> DR-matmul transpose -> scatter into padded
            # channel-major fp8 layout
            # ------------------------------------------------------------------
            xsT_tiles = {}

            def prep_group(img, g, b0g, xn_view, nblk, xsT):
                """sign+transpose+scatter for blocks [b0g, b0g+nblk)."""
                xg = xsg_p.tile([RB, PGROUP, C], FP8, tag="xg", name=f"xg{img}{g}")
                # chunk0 on DVE (+-0.5), chunk1 on ACT (+-1; w1 compensates)
                nc.vector.tensor_scalar(
                    xg[:, :nblk, 0:KP], xn_view[:, :, 0:KP], 0.0, 0.5,
                    Alu.is_ge, Alu.subtract,
                )
                nc.scalar.activation(
                    xg[:, :nblk, KP:C], xn_view[:, :, KP:C], Act.Sign,
                    bias=0.0, scale=1.0,
                )
                for t0 in range(0, nblk, TPB):
                    tn = min(TPB, nblk - t0)
                    tp = tp_p.tile(
                        [KP, TPB, 256], F32, tag="tp", name=f"tp{img}{g}{t0}"
                    )
                    for bi in range(tn):
                        lhsT = xg[:, t0 + bi, :].rearrange("p (k m) -> p k m", k=2)
                        nc.tensor.matmul(
                            tp[:, bi, 0 : 2 * RB],
                            lhsT,
                            ident[:RB],
                            start=True,
                            stop=True,
                            perf_mode=DR,
                        )
                    # scatter: tp[p, bi, RB*j + 56*r + c] -> xsT rows
                    b0 = b0g + t0
                    for j in range(NCHUNK):
                        src = tp[:, 0:tn, RB * j : RB * j + RB].rearrange(
                            "p b (r w) -> p b r w", w=W
                        )
                        a0 = DOFF + (1 + 2 * b0) * Wp
                        dst = xsT[:, j, a0 : a0 + 2 * tn * Wp].rearrange(
                            "p (b r w) -> p b r w", r=2, w=Wp
                        )[:, :, :, 1 : 1 + W]
                        if j == 0:
                            nc.scalar.copy(dst, src)
                        else:
                            nc.vector.tensor_copy(dst, src)

            def prep_input(img, pieces):
                """pieces: list of (xn_view_fn, nblk) DMA'd natural tiles."""
                xsT = xsT_p.tile(
                    [KP, NCHUNK, S_chunk], FP8, tag="xsT", name=f"xsT{img}"
                )
                border_memsets(xsT)
                g = 0
                for xn, nblk_piece in pieces:
                    off = 0
                    while off < nblk_piece:
                        nblk = min(PGROUP, nblk_piece - off)
                        prep_group(img, g, xn[:, off : off + nblk, :], nblk, xsT)
                        off += nblk
                        g += 1
                xsT_tiles[img] = xsT

            # ------------------------------------------------------------------
            # conv as 9 shifted DR matmuls per (stretch, chunk)
            # ------------------------------------------------------------------
            def conv_stretch(inbuf, wsb, si, psum_cb, cv):
                # per-row matmuls (56 wide) skip the 2 pad columns per row;
                # row 0 of tap 0 carries start (marks the whole psum zero
                # region), the last row of tap 8 carries stop
                r0, rg = STRETCHES[si]
                cs = (1 + r0) * Wp
                for j in range(NCHUNK):
                    ps = convp.tile([KP, 512], F32, tag="cv", name=f"{cv}{si}{j}")
                    for t in range(9):
                        dy, dx = t // 3, t % 3
                        off = (dy - 1) * Wp + (dx - 1)
                        a = DOFF + cs + off + 1
                        for r in range(rg):
                            nc.tensor.matmul(
                                ps[:, r * Wp + 1 : r * Wp + 1 + W],
                                wsb[:, t, j],
                                inbuf[:, :, a + r * Wp : a + r * Wp + W],
                                start=(t == 0 and r == 0),
                                stop=(t == 8 and r == rg - 1),
                                perf_mode=DR,
                            )
                    psum_cb(si, j, ps, r0, rg)

            onat_box = [None]

            def make_prep(img, pieces):
                """Allocate xsT(img) + memsets; return per-group emit closures."""
                xsT = xsT_p.tile(
                    [KP, NCHUNK, S_chunk], FP8, tag="xsT", name=f"xsT{img}"
                )
                border_memsets(xsT)
                closures = []
                g = 0
                b0g = 0
                for xn, nblk_piece in pieces:
                    off = 0
                    while off < nblk_piece:
                        nblk = min(PGROUP, nblk_piece - off)

                        def mk(g=g, b0g=b0g, xn=xn, off=off, nblk=nblk):
                            return lambda: prep_group(
                                img, g, b0g, xn[:, off : off + nblk, :], nblk, xsT
                            )

                        closures.append(mk())
                        off += nblk
                        b0g += nblk
                        g += 1
                xsT_tiles[img] = xsT
                return closures

            def conv1_emitters(img):
                xsT = xsT_tiles.pop(img)
                hsT = hsT_p.tile(
                    [KP, NCHUNK, S_chunk], FP8, tag="hsT", name=f"hsT{img}"
                )
                border_memsets(hsT)

                def bnsign(si, j, ps, r0, rg):
                    a0 = DOFF + (1 + r0) * Wp
                    dstv = hsT[:, j, a0 : a0 + rg * Wp].rearrange(
                        "p (r w) -> p r w", w=Wp
                    )[:, :, 1 : 1 + W]
                    srcv = ps[:, 0 : rg * Wp].rearrange("p (r w) -> p r w", w=Wp)[
                        :, :, 1 : 1 + W
                    ]
                    nc.scalar.activation(
                        dstv, srcv, Act.Sign, bias=nt1sb[:, j : j + 1], scale=1.0
                    )

                def emit(si):
                    conv_stretch(xsT, w1sb, si, bnsign, f"c1i{img}s")

                return hsT, emit

            def conv1_image(img, own_groups=None):
                hsT, emit = conv1_emitters(img)
                for si in range(len(STRETCHES)):
                    if own_groups and si in own_groups:
                        own_groups[si]()
                    emit(si)
                return hsT

            def conv2_emitters(img, hsT):
                if img == 0:
                    onat_box[0] = onat_p.tile(
                        [KP, 3, NCHUNK, PO], BF16, tag="on012", name="on012"
                    )
                if img == 3:
                    onat_box[0] = onat_p.tile(
                        [KP, 1, NCHUNK, PO], BF16, tag="on3", name="on3"
                    )
                onat = onat_box[0]
                slot = img if img < 3 else 0

                def pool_piece(si, j, ps, r0, rg, h0, hn):
                    # 2x2 maxpool rows [r0+2h0, r0+2h0+2hn) in one DVE
                    # reduce (one PSUM operand); fp16 holds ints exactly
                    q0 = r0 // 2 + h0
                    win = ps[:, 2 * h0 * Wp : (2 * h0 + 2 * hn) * Wp].rearrange(
                        "p (q r w) -> p q r w", r=2, w=Wp
                    )[:, :, :, 1 : 1 + W]
                    win = win.rearrange("p q r (u c) -> p q u r c", c=2)
                    pv = pv_p.tile(
                        [KP, 4, WH], FP16, tag="pv", bufs=2,
                        name=f"pv{img}{si}{j}{h0}",
                    )
                    nc.vector.tensor_reduce(
                        pv[:, 0:hn, :], win, mybir.AxisListType.XY, Alu.max
                    )
                    pvv = pv[:, 0:hn, :].rearrange("p q w -> p (q w)")
                    nc.vector.tensor_scalar(
                        onat[:, slot, j, q0 * WH : (q0 + hn) * WH],
                        pvv,
                        s2sb[:, j : j + 1],
                        b2sb[:, j : j + 1],
                        Alu.mult,
                        Alu.add,
                    )

                def pool(si, j, ps, r0, rg):
                    pool_piece(si, j, ps, r0, rg, 0, rg // 2)

                def emit(si):
                    conv_stretch(hsT, w2sb, si, pool, f"c2i{img}s")
                    if img == 3 and SPLIT_Y3 and si == len(STRETCHES) - 3:
                        r0l, rgl = STRETCHES[-2]
                        po_cut = (r0l // 2) * WH
                        dst = dram_ap(
                            y_h,
                            3 * NCHUNK * KP * PO,
                            [[PO, KP], [KP * PO, NCHUNK], [1, po_cut]],
                        )
                        nc.sync.dma_start(out=dst, in_=onat[:, 0, :, 0:po_cut])

                def finish():
                    if img == 2:
                        dst = dram_ap(
                            y_h,
                            0,
                            [
                                [PO, KP],
                                [NCHUNK * KP * PO, 3],
                                [KP * PO, NCHUNK],
                                [1, PO],
                            ],
                        )
                        nc.sync.dma_start(out=dst, in_=onat[:, :, :, :])
                    if img == 3:
                        po_cut = (
                            (STRETCHES[-2][0] // 2) * WH if SPLIT_Y3 else 0
                        )
                        for j in range(NCHUNK):
                            dst = dram_ap(
                                y_h,
                                3 * NCHUNK * KP * PO + j * KP * PO + po_cut,
                                [[PO, KP], [1, PO - po_cut]],
                            )
                            nc.sync.dma_start(
                                out=dst, in_=onat[:, 0, j, po_cut:PO]
                            )

                return emit, finish

            def conv2_image(img, hsT, next_groups):
                emit, finish = conv2_emitters(img, hsT)
                for si in range(len(STRETCHES)):
                    if si in next_groups:
                        next_groups[si]()
                    emit(si)
                for k in sorted(next_groups):
                    if k >= len(STRETCHES):
                        next_groups[k]()
                finish()

            # ------------------------------------------------------------------
            # emission: all DMAs up front; img0's prep interleaved into its own
            # conv1; prep(i+1) interleaved into conv2(i)
            # ------------------------------------------------------------------
            piece_tiles = []
            b0 = 0
            for pi, nblk in enumerate(X0_PIECES):
                t = xnat_p.tile(
                    [RB, nblk, C], F32, tag=f"xn0{pi}", name=f"xn0{pi}"
                )
                nc.sync.dma_start(
                    out=t,
                    in_=dram_ap(
                        x_h, b0 * RB * C, [[C, RB], [RB * C, nblk], [1, C]]
                    ),
                )
                piece_tiles.append((t, nblk))
                b0 += nblk
                if pi == 0:
                    load_consts1()
            load_consts2()
            xn1 = xnat_p.tile([RB, NB, C], F32, tag="xn1", name="xn1")
            nc.sync.dma_start(
                out=xn1,
                in_=dram_ap(x_h, H * W * C, [[C, RB], [RB * C, NB], [1, C]]),
            )
            xn23 = xnat_p.tile([RB, 2 * NB, C], F32, tag="xn23", name="xn23")
            nc.sync.dma_start(
                out=xn23,
                in_=dram_ap(
                    x_h, 2 * H * W * C, [[C, RB], [RB * C, 2 * NB], [1, C]]
                ),
            )
            xn_of = {1: xn1, 2: xn23[:, :NB, :], 3: xn23[:, NB:, :]}

            # conv1 stretch si needs sign-groups: s0,s1: g0; s2: g1; s3..s5: g2
            # (s3 reads row 28); s6,s7: g3
            # img0: interleave conv1/conv2 stretches (conv2 lags conv1 by
            # one stretch) so conv2 work fills the x-piece feed stalls
            prep0 = make_prep(0, piece_tiles)
            own0 = {0: prep0[0], 1: prep0[1], 3: prep0[2], 5: prep0[3]}
            hsT, c1emit = conv1_emitters(0)
            c2emit, c2fin = conv2_emitters(0, hsT)
            prep1 = make_prep(1, [(xn_of[1], NB)])
            ng1 = {3: prep1[0], 4: prep1[1], 5: prep1[2], 6: prep1[3]}
            NS = len(STRETCHES)
            for si in range(NS):
                if si in own0:
                    own0[si]()
                c1emit(si)
                if si >= C2LAG:
                    if (si - C2LAG) in ng1:
                        ng1[si - C2LAG]()
                    c2emit(si - C2LAG)
            for si in range(NS - C2LAG, NS):
                if si in ng1:
                    ng1[si]()
                c2emit(si)
            c2fin()
            hsT = conv1_image(1)
            for img in range(1, B):
                if img + 1 < B:
                    prepn = make_prep(img + 1, [(xn_of[img + 1], NB)])
                    ng = {2: prepn[0], 3: prepn[1], 5: prepn[2], 7: prepn[3]}
                else:
                    ng = {}
                conv2_image(img, hsT, ng)
                if img + 1 < B:
                    hsT = conv1_image(img + 1)

    nc.compile()
    return nc


# ---------------------------------------------------------------------------
# host-side constant prep
# ---------------------------------------------------------------------------


def _prep_consts(w1, beta1, mean1, var1, w2, beta2, mean2, var2):
    import jax
    import jax.numpy as jnp
    from jax import lax
    from concourse import mybir

    fp8np = mybir.dt.np(mybir.dt.float8e4)

    def prep_w(w, k0_scale, both=False):
        ws = np.where(np.asarray(w) >= 0, np.float32(1.0), np.float32(-1.0))
        # [3,3,ci,co] -> [p, tap, j, ktile, m]; ci = ktile*128+p, co = j*128+m
        wr = ws.reshape(9, 2, KP, NCHUNK, KP).transpose(2, 0, 3, 1, 4).copy()
        if both:
            wr *= k0_scale
        else:
            wr[:, :, :, 0, :] *= k0_scale  # compensate x-chunk0 magnitude
        return np.ascontiguousarray(wr).astype(fp8np)

    # conv1: x = +-0.5 everywhere -> scale all conv1 weights x2
    w1p = prep_w(w1, 2.0)
    # conv2: both chunks +-1
    w2p = prep_w(w2, 1.0)

    cpu = jax.devices("cpu")[0]
    MAXH = 9 * C
    with jax.default_device(cpu):
        hs = jnp.arange(-MAXH, MAXH + 1, dtype=jnp.float32)
        bn1 = (hs[:, None] - jnp.asarray(mean1)[None, :]) * lax.rsqrt(
            jnp.asarray(var1) + 1e-3
        )[None, :] + jnp.asarray(beta1)[None, :]
        nonneg = np.asarray(bn1 >= 0)
        r2 = np.asarray(lax.rsqrt(jnp.asarray(var2) + 1e-3))

    assert (np.diff(nonneg.astype(np.int8), axis=0) >= 0).all(), "bn1 not monotone"
    kc = np.where(nonneg.any(0), nonneg.argmax(0), 2 * MAXH + 1) - MAXH
    # device psum holds h1 exactly: sign flips at kc - 0.5
    nt1 = (-(kc.astype(np.float64) - 0.5)).astype(np.float32)

    s2 = r2.astype(np.float32)
    b2 = (
        np.asarray(beta2, np.float64)
        - np.asarray(mean2, np.float64) * s2.astype(np.float64)
    ).astype(np.float32)

    def to_pj(a):  # [256] -> [128, 2] with c = j*128+p
        return np.ascontiguousarray(a.reshape(NCHUNK, KP).T).astype(np.float32)

    cbuf = np.zeros((KP, CONST_B), dtype=np.uint8)

    def put(off, arr):
        by = np.ascontiguousarray(arr).reshape(KP, -1).view(np.uint8)
        cbuf[:, off : off + by.shape[1]] = by

    put(W1_OFF, w1p)
    put(W2_OFF, w2p)
    put(NT1_OFF, to_pj(nt1))
    put(S2_OFF, to_pj(s2))
    put(B2_OFF, to_pj(b2))
    return {"cb": cbuf}


# ---------------------------------------------------------------------------
# entry point
# ---------------------------------------------------------------------------

_cached = {}


def _get_program(Bc, H, W):
    key = (Bc, H, W)
    if key not in _cached:
        _cached[key] = build_program(Bc, H, W)
    return _cached[key]


def _unshard(res_list, Bt, H, W):
    # y per core: [Bc, 2, 128, PO] bf16 -> [Bt, H/2, W/2, C] f32
    y = np.concatenate(
        [np.asarray(r["y"]).astype(np.float32) for r in res_list], axis=0
    )
    y = y.transpose(0, 3, 1, 2).reshape(Bt, (H // 2) * (W // 2), C)
    return y.reshape(Bt, H // 2, W // 2, C)


def _run(inputs, trace=False):
    from concourse import bass_utils

    x = np.asarray(inputs["x"], dtype=np.float32)
    Bt, H, W, _ = x.shape  # 32, 56, 56, 256
    Bc = Bt // N_CORES

    consts = _prep_consts(
        inputs["w1"], inputs["beta1"], inputs["mean1"], inputs["var1"],
        inputs["w2"], inputs["beta2"], inputs["mean2"], inputs["var2"],
    )

    nc = _get_program(Bc, H, W)

    in_maps = []
    for c in range(N_CORES):
        m = dict(consts)
        m["x"] = np.ascontiguousarray(x[c * Bc : (c + 1) * Bc].reshape(Bc, H * W, C))
        in_maps.append(m)

    res = bass_utils.run_bass_kernel_spmd(
        nc, in_maps, core_ids=list(range(N_CORES)), trace=trace
    )
    y = _unshard(res.results, Bt, H, W)
    return y, res


def kernel(**inputs):
    y, _ = _run(inputs, trace=False)
    return y


# revision 6
# speedup vs baseline: 1.0464x; 1.0121x over previous
"""Trainium2 Bass kernel for a BinaryNet conv block.

Pipeline (per core, data-parallel over batch across 8 cores):
  sign(x) -> conv3x3(sign(w1)) -> BN1 -> sign -> conv3x3(sign(w2))
          -> maxpool2x2 -> BN2

Design notes (see TimelineSim cost model for the numbers):
  - Convs run as 9-tap shifted matmuls in fp8 DoubleRow (K=256/pass,
    0.5 cyc per output column). Matmuls are emitted per image row (56
    cols) so the 2 pad columns per 58-wide padded row are never
    computed; row 0 of tap 0 carries `start`, last row of tap 8 `stop`.
  - Activations are +-0.5 (chunk0, DVE tensor_scalar) / +-1 (chunk1, ACT
    Sign); w1's chunk-0 rows are pre-scaled x2 so conv1 PSUM = h1
    exactly, and BN1+sign folds into one ACT Sign against an integer
    threshold (bit-exact vs the fp32 reference).
  - Input transposes into the channel-major layout run as regular fp8
    DoubleRow matmuls: the signed 2-row block is the *stationary*
    operand and a block-diagonal [112, 2, 224] identity streams, giving
    0.5 cyc/col (half the cost of transpose-mode matmuls).
  - conv2 output is 2x2-maxpooled by a single DVE tensor_reduce
    (axis=XY) per stretch straight out of PSUM (engines may read only
    one PSUM operand per instruction), then BN2 via tensor_scalar into
    a bf16 channel-major output tile; the host does the final
    [b, c, s] -> [b, s, c] transpose and f32 upcast (bf16 rounding is
    ~3e-3 relative, well inside the 2e-2 gate).
  - Conv stretches are <=8 rows so each (stretch, chunk) PSUM group
    fits one 2KB bank; PSUM: 4 conv bufs + 2 transpose bufs = 8 banks.
  - Schedule: the Tile list-scheduler follows emission order among
    ready ops, so emission is the schedule: img0's x load is split in
    four pieces with sign/transpose/scatter groups interleaved into
    conv1 stretches, conv2(img0) lags conv1 by two stretches to fill
    the DMA feed stalls, and prep(img i+1) is interleaved into
    conv2(img i). Consts load in two pieces (w1 early, w2 late); the
    img3 output ships in three pieces so the tail only waits on the
    last 224 columns.
"""

import os
import numpy as np

os.environ.setdefault("MYCRO_LOCAL_CACHE", "1")

N_CORES = 8
C = 256
NCHUNK = 2
KP = 128

# packed consts layout (bytes per partition); cb1 = [0, CB1), cb2 = rest
W1_OFF = 0
NT1_OFF = 4608  # f32 [2]
S2_OFF = 4616
B2_OFF = 4624
CB1 = 4632
W2_OFF = 4632
CONST_B = 9240

# img0 x-load pieces, in blocks (2 rows each); must align to PGROUP
X0_PIECES = (7, 7, 7, 7)
USE_GPSIMD_CB = False  # consts via SWDGE (Pool-engine) DMA, off the HWDGE path
SPLIT_Y3 = True  # ship img3 output in two pieces to shorten the tail
WARMUP_MM = 45  # dummy PE matmuls to climb the p-state ramp during startup
PGROUP = 7  # sign-group size in blocks

# conv row-stretches (r0, rg): rg*Wp <= 512 psum bank; even rg for pooling
STRETCHES = [(0, 8), (8, 4), (12, 8), (20, 8), (28, 8), (36, 4), (40, 8), (48, 8)]


def build_program(B, H, W):
    """Build the per-core Bass program. B images of HxWxC per core."""
    import concourse.bass as bass
    import concourse.bacc as bacc
    import concourse.tile as tile
    from concourse import mybir, masks

    F32 = mybir.dt.float32
    FP16 = mybir.dt.float16
    BF16 = mybir.dt.bfloat16
    FP8 = mybir.dt.float8e4
    U8 = mybir.dt.uint8
    DR = mybir.MatmulPerfMode.DoubleRow
    Alu = mybir.AluOpType
    Act = mybir.ActivationFunctionType

    Hp, Wp = H + 2, W + 2
    DOFF = 32  # left zero pad inside each channel-chunk row buffer
    S_chunk = ((Hp * Wp + DOFF + 32 + 15) // 16) * 16
    RB = 2 * W  # transpose block = 2 image rows
    NB = H // 2  # blocks per image
    PO = (H // 2) * (W // 2)
    WH = W // 2
    HH = H // 2
    TPB = 4  # transpose-psum blocks per tile (512B slots -> bank aligned)

    assert sum(rg for _, rg in STRETCHES) == H
    assert all(rg * Wp <= 512 for _, rg in STRETCHES)
    assert all(rg % 2 == 0 for _, rg in STRETCHES)

    nc = bacc.Bacc("TRN2", target_bir_lowering=False, debug=False)

    x_h = nc.dram_tensor("x", [B, H * W, C], F32, kind="ExternalInput")
    cb_h = nc.dram_tensor("cb", [KP, CONST_B], U8, kind="ExternalInput")
    y_h = nc.dram_tensor("y", [B, NCHUNK, KP, PO], BF16, kind="ExternalOutput")

    def dram_ap(handle, offset, dims):
        return bass.AP(
            tensor=handle.ap().tensor, offset=offset, ap=[list(d) for d in dims]
        )

    with tile.TileContext(nc) as tc:
        from contextlib import ExitStack

        with ExitStack() as ctx:
            consts = ctx.enter_context(tc.tile_pool(name="consts", bufs=1))
            xnat_p = ctx.enter_context(tc.tile_pool(name="xnat", bufs=1))
            xsg_p = ctx.enter_context(tc.tile_pool(name="xsg", bufs=3))
            xsT_p = ctx.enter_context(tc.tile_pool(name="xsT", bufs=2))
            hsT_p = ctx.enter_context(tc.tile_pool(name="hsT", bufs=3))
            pv_p = ctx.enter_context(tc.tile_pool(name="pvp", bufs=2))
            onat_p = ctx.enter_context(tc.tile_pool(name="onat", bufs=1))
            convp = ctx.enter_context(tc.tile_pool(name="convp", bufs=4, space="PSUM"))
            tp_p = ctx.enter_context(tc.tile_pool(name="tpp", bufs=2, space="PSUM"))

            # --- identities for the DR-matmul transposes, built on GPSIMD
            ident = consts.tile([KP, NCHUNK, 2 * RB], FP8)
            nc.gpsimd.memset(ident, 0.0)
            for r in range(NCHUNK):
                masks.make_identity(
                    nc, ident[:RB, r, RB * r : RB * (r + 1)], nomemset=True
                )

            # --- packed constants: one DMA; bitcast views
            cb = consts.tile([KP, CONST_B], U8)

            def load_consts1():
                nc.sync.dma_start(
                    out=cb[:, 0:CB1],
                    in_=dram_ap(cb_h, 0, [[CONST_B, KP], [1, CB1]]),
                )

            def load_consts2():
                nc.sync.dma_start(
                    out=cb[:, CB1:CONST_B],
                    in_=dram_ap(cb_h, CB1, [[CONST_B, KP], [1, CONST_B - CB1]]),
                )

            w1sb = cb[:, W1_OFF : W1_OFF + 4608].bitcast(FP8).rearrange(
                "p (t j k m) -> p t j k m", t=9, j=NCHUNK, k=2
            )
            w2sb = cb[:, W2_OFF : W2_OFF + 4608].bitcast(FP8).rearrange(
                "p (t j k m) -> p t j k m", t=9, j=NCHUNK, k=2
            )
            nt1sb = cb[:, NT1_OFF : NT1_OFF + 8].bitcast(F32)
            s2sb = cb[:, S2_OFF : S2_OFF + 8].bitcast(F32)
            b2sb = cb[:, B2_OFF : B2_OFF + 8].bitcast(F32)

            def border_memsets(buf):
                nc.gpsimd.memset(buf[:, :, 0 : DOFF + Wp], 0.0)
                nc.gpsimd.memset(buf[:, :, DOFF + (H + 1) * Wp : S_chunk], 0.0)
                rows = buf[:, :, DOFF + Wp : DOFF + (H + 1) * Wp].rearrange(
                    "p j (r w) -> p j r w", w=Wp
                )
                nc.gpsimd.memset(rows[:, :, :, 0 :: (W + 1)], 0.0)

            # ------------------------------------------------------------------
            # input prep: sign -> DR-matmul transpose -> scatter into padded
            # channel-major fp8 layout
            # ------------------------------------------------------------------
            xsT_tiles = {}

            def prep_group(img, g, b0g, xn_view, nblk, xsT):
                """sign+transpose+scatter for blocks [b0g, b0g+nblk)."""
                xg = xsg_p.tile([RB, PGROUP, C], FP8, tag="xg", name=f"xg{img}{g}")
                # chunk0 on DVE (+-0.5), chunk1 on ACT (+-1; w1 compensates)
                nc.vector.tensor_scalar(
                    xg[:, :nblk, 0:KP], xn_view[:, :, 0:KP], 0.0, 0.5,
                    Alu.is_ge, Alu.subtract,
                )
                nc.scalar.activation(
                    xg[:, :nblk, KP:C], xn_view[:, :, KP:C], Act.Sign,
                    bias=0.0, scale=1.0,
                )
                for t0 in range(0, nblk, TPB):
                    tn = min(TPB, nblk - t0)
                    tp = tp_p.tile(
                        [KP, TPB, 256], F32, tag="tp", name=f"tp{img}{g}{t0}"
                    )
                    for bi in range(tn):
                        lhsT = xg[:, t0 + bi, :].rearrange("p (k m) -> p k m", k=2)
                        nc.tensor.matmul(
                            tp[:, bi, 0 : 2 * RB],
                            lhsT,
                            ident[:RB],
                            start=True,
                            stop=True,
                            perf_mode=DR,
                        )
                    # scatter: tp[p, bi, RB*j + 56*r + c] -> xsT rows
                    b0 = b0g + t0
                    for j in range(NCHUNK):
                        src = tp[:, 0:tn, RB * j : RB * j + RB].rearrange(
                            "p b (r w) -> p b r w", w=W
                        )
                        a0 = DOFF + (1 + 2 * b0) * Wp
                        dst = xsT[:, j, a0 : a0 + 2 * tn * Wp].rearrange(
                            "p (b r w) -> p b r w", r=2, w=Wp
                        )[:, :, :, 1 : 1 + W]
                        if j == 0:
                            nc.scalar.copy(dst, src)
                        else:
                            nc.vector.tensor_copy(dst, src)

            def prep_input(img, pieces):
                """pieces: list of (xn_view_fn, nblk) DMA'd natural tiles."""
                xsT = xsT_p.tile(
                    [KP, NCHUNK, S_chunk], FP8, tag="xsT", name=f"xsT{img}"
                )
                border_memsets(xsT)
                g = 0
                for xn, nblk_piece in pieces:
                    off = 0
                    while off < nblk_piece:
                        nblk = min(PGROUP, nblk_piece - off)
                        prep_group(img, g, xn[:, off : off + nblk, :], nblk, xsT)
                        off += nblk
                        g += 1
                xsT_tiles[img] = xsT

            # ------------------------------------------------------------------
            # conv as 9 shifted DR matmuls per (stretch, chunk)
            # ------------------------------------------------------------------
            def conv_stretch(inbuf, wsb, si, psum_cb, cv):
                # per-row matmuls (56 wide) skip the 2 pad columns per row;
                # row 0 of tap 0 carries start (marks the whole psum zero
                # region), the last row of tap 8 carries stop
                r0, rg = STRETCHES[si]
                cs = (1 + r0) * Wp
                for j in range(NCHUNK):
                    ps = convp.tile([KP, 512], F32, tag="cv", name=f"{cv}{si}{j}")
                    for t in range(9):
                        dy, dx = t // 3, t % 3
                        off = (dy - 1) * Wp + (dx - 1)
                        a = DOFF + cs + off + 1
                        for r in range(rg):
                            nc.tensor.matmul(
                                ps[:, r * Wp + 1 : r * Wp + 1 + W],
                                wsb[:, t, j],
                                inbuf[:, :, a + r * Wp : a + r * Wp + W],
                                start=(t == 0 and r == 0),
                                stop=(t == 8 and r == rg - 1),
                                perf_mode=DR,
                            )
                    psum_cb(si, j, ps, r0, rg)

            onat_box = [None]

            def make_prep(img, pieces):
                """Allocate xsT(img) + memsets; return per-group emit closures."""
                xsT = xsT_p.tile(
                    [KP, NCHUNK, S_chunk], FP8, tag="xsT", name=f"xsT{img}"
                )
                border_memsets(xsT)
                closures = []
                g = 0
                b0g = 0
                for xn, nblk_piece in pieces:
                    off = 0
                    while off < nblk_piece:
                        nblk = min(PGROUP, nblk_piece - off)

                        def mk(g=g, b0g=b0g, xn=xn, off=off, nblk=nblk):
                            return lambda: prep_group(
                                img, g, b0g, xn[:, off : off + nblk, :], nblk, xsT
                            )

                        closures.append(mk())
                        off += nblk
                        b0g += nblk
                        g += 1
                xsT_tiles[img] = xsT
                return closures

            def conv1_emitters(img):
                xsT = xsT_tiles.pop(img)
                hsT = hsT_p.tile(
                    [KP, NCHUNK, S_chunk], FP8, tag="hsT", name=f"hsT{img}"
                )
                border_memsets(hsT)

                def bnsign(si, j, ps, r0, rg):
                    a0 = DOFF + (1 + r0) * Wp
                    dstv = hsT[:, j, a0 : a0 + rg * Wp].rearrange(
                        "p (r w) -> p r w", w=Wp
                    )[:, :, 1 : 1 + W]
                    srcv = ps[:, 0 : rg * Wp].rearrange("p (r w) -> p r w", w=Wp)[
                        :, :, 1 : 1 + W
                    ]
                    nc.scalar.activation(
                        dstv, srcv, Act.Sign, bias=nt1sb[:, j : j + 1], scale=1.0
                    )

                def emit(si):
                    conv_stretch(xsT, w1sb, si, bnsign, f"c1i{img}s")

                return hsT, emit

            def conv1_image(img, own_groups=None):
                hsT, emit = conv1_emitters(img)
                for si in range(len(STRETCHES)):
                    if own_groups and si in own_groups:
                        own_groups[si]()
                    emit(si)
                return hsT

            def conv2_emitters(img, hsT):
                if img == 0:
                    onat_box[0] = onat_p.tile(
                        [KP, 3, NCHUNK, PO], BF16, tag="on012", name="on012"
                    )
                if img == 3:
                    onat_box[0] = onat_p.tile(
                        [KP, 1, NCHUNK, PO], BF16, tag="on3", name="on3"
                    )
                onat = onat_box[0]
                slot = img if img < 3 else 0

                def pool_piece(si, j, ps, r0, rg, h0, hn):
                    # 2x2 maxpool rows [r0+2h0, r0+2h0+2hn) in one DVE
                    # reduce (one PSUM operand); fp16 holds ints exactly
                    q0 = r0 // 2 + h0
                    win = ps[:, 2 * h0 * Wp : (2 * h0 + 2 * hn) * Wp].rearrange(
                        "p (q r w) -> p q r w", r=2, w=Wp
                    )[:, :, :, 1 : 1 + W]
                    win = win.rearrange("p q r (u c) -> p q u r c", c=2)
                    pv = pv_p.tile(
                        [KP, 4, WH], FP16, tag="pv", bufs=3,
                        name=f"pv{img}{si}{j}{h0}",
                    )
                    nc.vector.tensor_reduce(
                        pv[:, 0:hn, :], win, mybir.AxisListType.XY, Alu.max
                    )
                    pvv = pv[:, 0:hn, :].rearrange("p q w -> p (q w)")
                    nc.vector.tensor_scalar(
                        onat[:, slot, j, q0 * WH : (q0 + hn) * WH],
                        pvv,
                        s2sb[:, j : j + 1],
                        b2sb[:, j : j + 1],
                        Alu.mult,
                        Alu.add,
                    )

                def pool(si, j, ps, r0, rg):
                    pool_piece(si, j, ps, r0, rg, 0, rg // 2)

                def emit(si):
                    conv_stretch(hsT, w2sb, si, pool, f"c2i{img}s")
                    if img == 3 and SPLIT_Y3 and si == len(STRETCHES) - 3:
                        r0l, rgl = STRETCHES[-2]
                        po_cut = (r0l // 2) * WH
                        dst = dram_ap(
                            y_h,
                            3 * NCHUNK * KP * PO,
                            [[PO, KP], [KP * PO, NCHUNK], [1, po_cut]],
                        )
                        nc.sync.dma_start(out=dst, in_=onat[:, 0, :, 0:po_cut])

                def finish():
                    if img == 2:
                        dst = dram_ap(
                            y_h,
                            0,
                            [
                                [PO, KP],
                                [NCHUNK * KP * PO, 3],
                                [KP * PO, NCHUNK],
                                [1, PO],
                            ],
                        )
                        nc.sync.dma_start(out=dst, in_=onat[:, :, :, :])
                    if img == 3:
                        po_cut = (
                            (STRETCHES[-2][0] // 2) * WH if SPLIT_Y3 else 0
                        )
                        for j in range(NCHUNK):
                            dst = dram_ap(
                                y_h,
                                3 * NCHUNK * KP * PO + j * KP * PO + po_cut,
                                [[PO, KP], [1, PO - po_cut]],
                            )
                            nc.sync.dma_start(
                                out=dst, in_=onat[:, 0, j, po_cut:PO]
                            )

                return emit, finish

            def conv2_image(img, hsT, next_groups):
                emit, finish = conv2_emitters(img, hsT)
                for si in range(len(STRETCHES)):
                    if si in next_groups:
                        next_groups[si]()
                    emit(si)
                for k in sorted(next_groups):
                    if k >= len(STRETCHES):
                        next_groups[k]()
                finish()

            # ------------------------------------------------------------------
            # emission: all DMAs up front; img0's prep interleaved into its own
            # conv1; prep(i+1) interleaved into conv2(i)
            # ------------------------------------------------------------------
            piece_tiles = []
            b0 = 0
            for pi, nblk in enumerate(X0_PIECES):
                t = xnat_p.tile(
                    [RB, nblk, C], F32, tag=f"xn0{pi}", name=f"xn0{pi}"
                )
                nc.sync.dma_start(
                    out=t,
                    in_=dram_ap(
                        x_h, b0 * RB * C, [[C, RB], [RB * C, nblk], [1, C]]
                    ),
                )
                piece_tiles.append((t, nblk))
                b0 += nblk
                if pi == 0:
                    load_consts1()
            load_consts2()
            xn1 = xnat_p.tile([RB, NB, C], F32, tag="xn1", name="xn1")
            nc.sync.dma_start(
                out=xn1[:, :PGROUP, :],
                in_=dram_ap(x_h, H * W * C, [[C, RB], [RB * C, PGROUP], [1, C]]),
            )
            nc.sync.dma_start(
                out=xn1[:, PGROUP:, :],
                in_=dram_ap(
                    x_h, H * W * C + PGROUP * RB * C,
                    [[C, RB], [RB * C, NB - PGROUP], [1, C]],
                ),
            )
            xn23 = xnat_p.tile([RB, 2 * NB, C], F32, tag="xn23", name="xn23")
            nc.sync.dma_start(
                out=xn23,
                in_=dram_ap(
                    x_h, 2 * H * W * C, [[C, RB], [RB * C, 2 * NB], [1, C]]
                ),
            )
            xn_of = {1: xn1, 2: xn23[:, :NB, :], 3: xn23[:, NB:, :]}

            # conv1 stretch si needs sign-groups: s0,s1: g0; s2: g1; s3..s5: g2
            # (s3 reads row 28); s6,s7: g3
            # img0: interleave conv1/conv2 stretches (conv2 lags conv1 by
            # one stretch) so conv2 work fills the x-piece feed stalls
            prep0 = make_prep(0, piece_tiles)
            own0 = {0: [prep0[0]], 1: [prep0[1]], 3: [prep0[2]], 5: [prep0[3]]}
            hsT, c1emit = conv1_emitters(0)
            c2emit, c2fin = conv2_emitters(0, hsT)
            prep1 = make_prep(1, [(xn_of[1], NB)])
            ng1 = {3: prep1[0], 4: prep1[1], 5: prep1[2], 7: prep1[3]}
            NS = len(STRETCHES)
            for si in range(NS):
                for fn in own0.get(si, []):
                    fn()
                c1emit(si)
                if si >= C2LAG:
                    if (si - C2LAG) in ng1:
                        ng1[si - C2LAG]()
                    c2emit(si - C2LAG)
            for si in range(NS - C2LAG, NS):
                if si in ng1:
                    ng1[si]()
                c2emit(si)
            for k in sorted(ng1):
                if k >= NS:
                    ng1[k]()
            c2fin()
            # steady images: same lag-2 conv1/conv2 weave, prep(i+1)
            # interleaved at NGK positions of the conv2 sequence
            for img in range(1, B):
                hsTn, c1e = conv1_emitters(img)
                if img + 1 < B:
                    prepn = make_prep(img + 1, [(xn_of[img + 1], NB)])
                    ngn = {k: [prepn[i]] for i, k in enumerate(NGK)}
                else:
                    ngn = {}
                c2e, c2f = conv2_emitters(img, hsTn)
                for si in range(NS):
                    c1e(si)
                    if si >= C2LAG:
                        for fn in ngn.get(si - C2LAG, []):
                            fn()
                        c2e(si - C2LAG)
                for si in range(NS - C2LAG, NS):
                    for fn in ngn.get(si, []):
                        fn()
                    c2e(si)
                for k in sorted(ngn):
                    if k >= NS:
                        for fn in ngn[k]:
                            fn()
                c2f()

    nc.compile()
    return nc


# ---------------------------------------------------------------------------
# host-side constant prep
# ---------------------------------------------------------------------------


def _prep_consts(w1, beta1, mean1, var1, w2, beta2, mean2, var2):
    import jax
    import jax.numpy as jnp
    from jax import lax
    from concourse import mybir

    fp8np = mybir.dt.np(mybir.dt.float8e4)

    def prep_w(w, k0_scale, both=False):
        ws = np.where(np.asarray(w) >= 0, np.float32(1.0), np.float32(-1.0))
        # [3,3,ci,co] -> [p, tap, j, ktile, m]; ci = ktile*128+p, co = j*128+m
        wr = ws.reshape(9, 2, KP, NCHUNK, KP).transpose(2, 0, 3, 1, 4).copy()
        if both:
            wr *= k0_scale
        else:
            wr[:, :, :, 0, :] *= k0_scale  # compensate x-chunk0 magnitude
        return np.ascontiguousarray(wr).astype(fp8np)

    # conv1: x = +-0.5 everywhere -> scale all conv1 weights x2
    w1p = prep_w(w1, 2.0)
    # conv2: both chunks +-1
    w2p = prep_w(w2, 1.0)

    cpu = jax.devices("cpu")[0]
    MAXH = 9 * C
    with jax.default_device(cpu):
        hs = jnp.arange(-MAXH, MAXH + 1, dtype=jnp.float32)
        bn1 = (hs[:, None] - jnp.asarray(mean1)[None, :]) * lax.rsqrt(
            jnp.asarray(var1) + 1e-3
        )[None, :] + jnp.asarray(beta1)[None, :]
        nonneg = np.asarray(bn1 >= 0)
        r2 = np.asarray(lax.rsqrt(jnp.asarray(var2) + 1e-3))

    assert (np.diff(nonneg.astype(np.int8), axis=0) >= 0).all(), "bn1 not monotone"
    kc = np.where(nonneg.any(0), nonneg.argmax(0), 2 * MAXH + 1) - MAXH
    # device psum holds h1 exactly: sign flips at kc - 0.5
    nt1 = (-(kc.astype(np.float64) - 0.5)).astype(np.float32)

    s2 = r2.astype(np.float32)
    b2 = (
        np.asarray(beta2, np.float64)
        - np.asarray(mean2, np.float64) * s2.astype(np.float64)
    ).astype(np.float32)

    def to_pj(a):  # [256] -> [128, 2] with c = j*128+p
        return np.ascontiguousarray(a.reshape(NCHUNK, KP).T).astype(np.float32)

    cbuf = np.zeros((KP, CONST_B), dtype=np.uint8)

    def put(off, arr):
        by = np.ascontiguousarray(arr).reshape(KP, -1).view(np.uint8)
        cbuf[:, off : off + by.shape[1]] = by

    put(W1_OFF, w1p)
    put(W2_OFF, w2p)
    put(NT1_OFF, to_pj(nt1))
    put(S2_OFF, to_pj(s2))
    put(B2_OFF, to_pj(b2))
    return {"cb": cbuf}


# ---------------------------------------------------------------------------
# entry point
# ---------------------------------------------------------------------------

_cached = {}


def _get_program(Bc, H, W):
    key = (Bc, H, W)
    if key not in _cached:
        _cached[key] = build_program(Bc, H, W)
    return _cached[key]


def _unshard(res_list, Bt, H, W):
    # y per core: [Bc, 2, 128, PO] bf16 -> [Bt, H/2, W/2, C] f32
    y = np.concatenate(
        [np.asarray(r["y"]).astype(np.float32) for r in res_list], axis=0
    )
    y = y.transpose(0, 3, 1, 2).reshape(Bt, (H // 2) * (W // 2), C)
    return y.reshape(Bt, H // 2, W // 2, C)


def _run(inputs, trace=False):
    from concourse import bass_utils

    x = np.asarray(inputs["x"], dtype=np.float32)
    Bt, H, W, _ = x.shape  # 32, 56, 56, 256
    Bc = Bt // N_CORES

    consts = _prep_consts(
        inputs["w1"], inputs["beta1"], inputs["mean1"], inputs["var1"],
        inputs["w2"], inputs["beta2"], inputs["mean2"], inputs["var2"],
    )

    nc = _get_program(Bc, H, W)

    in_maps = []
    for c in range(N_CORES):
        m = dict(consts)
        m["x"] = np.ascontiguousarray(x[c * Bc : (c + 1) * Bc].reshape(Bc, H * W, C))
        in_maps.append(m)

    res = bass_utils.run_bass_kernel_spmd(
        nc, in_maps, core_ids=list(range(N_CORES)), trace=trace
    )
    y = _unshard(res.results, Bt, H, W)
    return y, res


def kernel(**inputs):
    y, _ = _run(inputs, trace=False)
    return y
